# revision 1
# baseline (speedup 1.0000x reference)
"""Trainium2 Bass kernel for nn_Block_73443940761664 (moe_routing).

Transformer block: LN1 -> causal MHA -> residual -> LN2 -> top-2-of-8
sparse MoE (dense-equivalent combine) -> residual.

Distribution over 8 NeuronCores:
  dispatch 1: attention head-parallel (2 heads/core); per-512-token-group
              ReduceScatter (bf16) of the Wo partial sums overlapped with the
              next group's compute; LN2/router on each core's 256-token
              shard; outputs a, h2 (normalized, bf16), router gates.
  host:       top-2 routing -> per-expert token gather (capacity-padded).
  dispatch 2: expert-parallel FFN (1 expert/core) on gathered tokens:
              phase 1 (h2 @ W1) in fp8e4m3 DoubleRow, phase 2 (mid @ W2)
              with fp8e3m4 weights and bf16 moving data; scaled by gate
              weight; host scatter-adds into the output.

LayerNorm gains/biases and all projection biases are folded into weights /
the residual stream on the host (pure preprocessing), so the device only
computes normalization itself.  The router product runs in true fp32 so
top-2 selection margins (min observed 2.6e-5) survive; everything else is
bf16/fp8 with fp32 accumulation.
"""

import numpy as np
import ml_dtypes

import concourse.tile as tile
import concourse.mybir as mybir
from concourse import bacc

P = 128
S = 2048
D = 1024
HD = 64           # head dim
HPC = 2           # heads per core
E = 8
F = 4096
N_CORES = 8
FP = mybir.dt.float32
FPR = mybir.dt.float32r
BF = mybir.dt.bfloat16
F8 = mybir.dt.float8e4    # e4m3 (DoubleRow capable)
F83 = mybir.dt.float8e3   # e3m4 (more mantissa, bf16-speed matmul)
AF = mybir.ActivationFunctionType
ALU = mybir.AluOpType
DR = mybir.MatmulPerfMode.DoubleRow
EPS = 1e-5

NT = S // P       # 16 token tiles
NG = S // 512     # 4 token groups of 512
ND = D // P       # 8 d-chunks
NF = F // P       # 32 f-tiles
SSH = S // N_CORES  # 256 tokens per core shard

NP_BF16 = ml_dtypes.bfloat16
W1S = 64.0   # host premultiplies W1; device applies 1/W1S in the gelu
W2S = 64.0   # host premultiplies W2; gates carry 1/W2S
NP_F8 = ml_dtypes.float8_e4m3
NP_F83 = ml_dtypes.float8_e3m4


def _layer_norm_tile(nc, eps_ap, x_ap, out_tile, stats_pool):
    """out = (x - mean)/sqrt(var+eps); x [128, D] sbuf -> out (any dtype)."""
    st = stats_pool.tile([P, 12], FP, tag="st")
    nc.vector.bn_stats(st[:, 0:6], x_ap[:, 0:512])
    nc.vector.bn_stats(st[:, 6:12], x_ap[:, 512:1024])
    mv = stats_pool.tile([P, 2], FP, tag="mv")
    nc.vector.bn_aggr(mv[:], st[:].rearrange("p (a b) -> p a b", a=2))
    std = stats_pool.tile([P, 1], FP, tag="std")
    nc.scalar.activation(std[:], mv[:, 1:2], AF.Sqrt, bias=eps_ap)
    rstd = stats_pool.tile([P, 1], FP, tag="rstd")
    nc.vector.reciprocal(rstd[:], std[:])
    nmrs = stats_pool.tile([P, 1], FP, tag="nmrs")
    nc.vector.scalar_tensor_tensor(
        out=nmrs[:], in0=mv[:, 0:1], scalar=-1.0, in1=rstd[:],
        op0=ALU.mult, op1=ALU.mult,
    )
    nc.scalar.activation(out_tile[:], x_ap[:], AF.Identity,
                         bias=nmrs[:], scale=rstd[:])
    return rstd, nmrs


def _s1_setup(nc, env):
    """Load weights/constants into SBUF; populate env."""
    misc_pool = env["misc"]
    eps_sb = misc_pool.tile([P, 1], FP)
    nc.vector.memset(eps_sb[:], EPS)
    # packed fp32r constants: [idenr | wqkv d0..d7 | wo | tri | onesd]
    NWR = P + ND * 3 * P + D + P + 1
    wrk = misc_pool.tile([P, NWR], FPR)
    nc.scalar.dma_start(wrk[:], env["wpack_r"][:])
    idenr_sb = wrk[:, 0:P]
    wqkv_sb = [wrk[:, P + d * 3 * P:P + (d + 1) * 3 * P] for d in range(ND)]
    o0 = P + ND * 3 * P
    wo_sb = wrk[:, o0:o0 + D]
    tri_sb = wrk[:, o0 + D:o0 + D + P]
    onesd_sb = wrk[:, NWR - 1:NWR]          # value 1/1024
    # packed fp32 constants: [iden | wr d0..d7]
    NWF = P + ND * E
    wf = misc_pool.tile([P, NWF], FP)
    nc.scalar.dma_start(wf[:], env["wpack_f"][:])
    iden_sb = wf[:, 0:P]
    wr_sb = [wf[:, P + d * E:P + (d + 1) * E] for d in range(ND)]
    corr2_sb = misc_pool.tile([2, 3 * P], FPR)
    nc.sync.dma_start(corr2_sb[:], env["corr"][:])
    srows_sb = misc_pool.tile([2, S], FPR)
    nc.sync.dma_start(srows_sb[:], env["srows_in"][:])
    rstdf_sb = misc_pool.tile([1, S], FP)
    nc.sync.dma_start(rstdf_sb[:], env["rstd_in"][:])
    brr_sb = misc_pool.tile([1, E], FP)
    nc.sync.dma_start(brr_sb[:], env["brr"][:])
    csw_sb = misc_pool.tile([1, E], FP)
    nc.sync.dma_start(csw_sb[:], env["csw"][:])
    brr_bc = misc_pool.tile([P, E], FP)
    nc.gpsimd.partition_broadcast(brr_bc[:], brr_sb[:])
    csw_bc = misc_pool.tile([P, E], FP)
    nc.gpsimd.partition_broadcast(csw_bc[:], csw_sb[:])

    qT_sb = env["qkvT"].tile([P, S], FPR)   # rows: h0 0:64 | h1 64:128
    kT_sb = env["qkvT"].tile([P, S], FPR)
    vT_sb = env["qkvT"].tile([P, S], FPR)
    # v_sb[kb]: [tok, 130] = [h0 v 0:64 | ones 64 | h1 v 65:129 | ones 129]
    ones_f = misc_pool.tile([P, 1], FP)
    nc.vector.memset(ones_f[:], 1.0)
    v_sb = []
    for kb in range(NT):
        vkb = env["vtile"].tile([P, 2 * (HD + 1)], FPR, tag=f"v{kb}")
        for h in range(HPC):
            nc.scalar.activation(vkb[:, h * (HD + 1) + HD:
                                     h * (HD + 1) + HD + 1],
                                 ones_f[:], AF.Identity)
        v_sb.append(vkb)
    env.update(eps_sb=eps_sb, iden_sb=iden_sb, idenr_sb=idenr_sb,
               wqkv_sb=wqkv_sb, wo_sb=wo_sb, tri_sb=tri_sb, wr_sb=wr_sb,
               onesd_sb=onesd_sb, corr2_sb=corr2_sb,
               srows_sb=srows_sb, rstdf_sb=rstdf_sb,
               brr_bc=brr_bc, csw_bc=csw_bc,
               qT_sb=qT_sb, kT_sb=kT_sb, vT_sb=vT_sb, v_sb=v_sb)


def _s1_qkv_gen(nc, env, g, xg_sb):
    """Generator emitting group g's QKV work in small chunks.

    Yields between chunks so the caller can interleave these PE/DVE/Pool
    instructions into attention(g-1)'s kb loop, filling exp-latency bubbles.
    """
    xT_g = xg_sb[g]
    col = slice(g * 512, (g + 1) * 512)
    srow_g = env["srows_sb"][:, col]          # [2, 512]: [std; -m]
    rstd_bc = env["srows"].tile([P, 512], FP, tag="rstdbc")
    nc.gpsimd.partition_broadcast(rstd_bc[:], env["rstdf_sb"][:, col])
    yield
    for which, dst in ((0, env["qT_sb"]), (1, env["kT_sb"]),
                       (2, env["vT_sb"])):
        ps = env["ps_qkv"].tile([P, 512], FP, tag=f"qkv{which}")
        wcol = slice(which * P, (which + 1) * P)
        for d in range(ND):
            nc.tensor.matmul(
                ps[:], env["wqkv_sb"][d][:, wcol], xT_g[:, d, :],
                start=(d == 0), stop=False)
            if d % 2 == 1:
                yield
        nc.tensor.matmul(ps[:], env["corr2_sb"][:, wcol], srow_g,
                         start=False, stop=True)
        nc.vector.tensor_mul(dst[:, col], ps[:], rstd_bc[:])
        yield
    # v transposed to [tok, dim] per key tile via PE
    for ti in range(4):
        kb = g * 4 + ti
        for h in range(HPC):
            ps = env["ps_rtr"].tile([P, HD], FPR, tag="rtr")
            nc.tensor.transpose(
                ps[:], env["vT_sb"][h * HD:(h + 1) * HD,
                                    kb * P:(kb + 1) * P],
                env["idenr_sb"][h * HD:(h + 1) * HD, h * HD:(h + 1) * HD])
            base = h * (HD + 1)
            nc.scalar.activation(env["v_sb"][kb][:, base:base + HD], ps[:],
                                 AF.Identity)
            yield


def _s1_group_attn(nc, env, g, a_part, filler=None):
    """Causal attention + Wo partial for group g; write a_part tiles.
    ``filler()`` emits one chunk of next-group QKV work per kb step."""
    def fill():
        if filler is not None:
            next(filler, None)
    qcol = slice(g * 512, (g + 1) * 512)
    oT_sb = env["oT"].tile([P, 512], FPR, tag="oT")
    for h in range(HPC):
        acc = env["ps_acc"].tile([HD + 1, 512], FP, tag="acc")
        nkb = g * 4 + 4
        base = h * (HD + 1)
        # diagonal tiles first (their extra mask hop overlaps later work);
        # the first diagonal tile covers the full 512 columns, so the
        # accumulation region is fully initialized by the start flag.
        order = list(range(g * 4, nkb)) + list(range(0, g * 4))
        for i, kb in enumerate(order):
            j = kb - g * 4          # >= 0 on the diagonal group
            c0 = max(0, j) * P      # first unmasked query column
            cs = slice(c0, 512)
            qs = slice(g * 512 + c0, (g + 1) * 512)
            sc = env["ps_mm"].tile([P, 512], FP, tag="mm")
            nc.tensor.matmul(
                sc[:, cs], env["kT_sb"][h * HD:(h + 1) * HD,
                                        kb * P:(kb + 1) * P],
                env["qT_sb"][h * HD:(h + 1) * HD, qs],
                start=True, stop=True)
            et = env["expT"].tile([P, 512], FPR, tag="exp")
            nc.scalar.activation(et[:, cs], sc[:, cs], AF.Exp, scale=0.125)
            if j >= 0:
                # triangular mask only on the 128-wide diagonal block
                nc.vector.tensor_mul(et[:, c0:c0 + P], et[:, c0:c0 + P],
                                     env["tri_sb"][:])
            nc.tensor.matmul(
                acc[:, cs], env["v_sb"][kb][:, base:base + HD + 1],
                et[:, cs],
                start=(i == 0), stop=(i == nkb - 1))
            fill()
        rec = env["small"].tile([1, 512], FP, tag="rec")
        nc.vector.reciprocal(rec[:], acc[HD:HD + 1, :])
        rec_bc = env["small"].tile([HD, 512], FP, tag="recbc")
        nc.gpsimd.partition_broadcast(rec_bc[:], rec[:])
        nc.vector.tensor_mul(oT_sb[h * HD:(h + 1) * HD, :],
                             acc[0:HD, :], rec_bc[:])
    for ti in range(4):
        t_idx = g * 4 + ti
        asb = env["aout"].tile([P, D], FP, tag="a")
        for half in range(2):
            colh = slice(half * 512, (half + 1) * 512)
            ps = env["ps_mm"].tile([P, 512], FP, tag="mm")
            nc.tensor.matmul(ps[:], oT_sb[:, ti * P:(ti + 1) * P],
                             env["wo_sb"][:, colh], start=True, stop=True)
            if half == 0:
                nc.scalar.activation(asb[:, colh], ps[:], AF.Identity)
            else:
                nc.vector.tensor_copy(asb[:, colh], ps[:])
        nc.sync.dma_start(a_part[t_idx * P:(t_idx + 1) * P, :], asb[:])


def _s1_post_tile(nc, env, tt, rs_out, xsh, a_shard, h2_shard, gates_shard):
    """a = rs + xsh; LN2; fp32 router; top-2 gates for shard tile tt."""
    rst = env["post"].tile([P, D], FP, tag="rst")
    nc.sync.dma_start(rst[:], rs_out[tt * P:(tt + 1) * P, :])
    xt = env["post"].tile([P, D], FP, tag="xsh")
    nc.sync.dma_start(xt[:], xsh[tt * P:(tt + 1) * P, :])
    at = env["aout"].tile([P, D], FP, tag="at")
    nc.vector.tensor_add(at[:], rst[:], xt[:])
    nc.sync.dma_start(a_shard[tt * P:(tt + 1) * P, :], at[:])
    h2t = env["hp"].tile([P, D], BF, tag="h")
    rstd2, nmrs2 = _layer_norm_tile(nc, env["eps_sb"][:], at[:], h2t,
                                    env["stats"])
    nc.sync.dma_start(h2_shard[tt * P:(tt + 1) * P, :], h2t[:])
    # true-fp32 router product: rawT = Wr_f.T @ a^T
    lps = env["ps_mm"].tile([E, P], FP, tag="mm")
    for d in range(ND):
        ps = env["ps_rtr"].tile([P, P], FP, tag="rtr")
        nc.tensor.transpose(ps[:], at[:, d * P:(d + 1) * P],
                            env["iden_sb"][:])
        aT = env["hp"].tile([P, P], FP, tag="aT")
        nc.vector.tensor_copy(aT[:], ps[:])
        nc.tensor.matmul(lps[:], env["wr_sb"][d][:], aT[:],
                         start=(d == 0), stop=(d == ND - 1))
    ltr = env["small"].tile([E, P], FP, tag="ltr")
    nc.scalar.activation(ltr[:], lps[:], AF.Identity)
    tps = env["ps_rtr"].tile([P, E], FP, tag="rtr")
    nc.tensor.transpose(tps[:], ltr[:], env["iden_sb"][0:E, 0:E])
    # token-major LN2 affine fold: logits = rstd*(a@Wr) + nmrs*csw + br
    small = env["small"]
    ltm = small.tile([P, E], FP, tag="ltmsb")
    nc.scalar.activation(ltm[:], tps[:], AF.Identity, scale=rstd2[:])
    nc.vector.scalar_tensor_tensor(
        out=ltm[:], in0=env["csw_bc"][:], scalar=nmrs2[:], in1=ltm[:],
        op0=ALU.mult, op1=ALU.add)
    nc.vector.tensor_add(ltm[:], ltm[:], env["brr_bc"][:])
    # top-2 softmax gates
    m1 = small.tile([P, 1], FP, tag="m1")
    nc.vector.tensor_reduce(m1[:], ltm[:], mybir.AxisListType.X, ALU.max)
    nm1 = small.tile([P, 1], FP, tag="nm1")
    nc.vector.tensor_scalar_mul(nm1[:], m1[:], -1.0)
    ex = small.tile([P, E], FP, tag="ex")
    nc.scalar.activation(ex[:], ltm[:], AF.Exp, bias=nm1[:])
    eq = small.tile([P, E], FP, tag="eq")
    nc.vector.tensor_scalar(out=eq[:], in0=ltm[:], scalar1=m1[:],
                            scalar2=None, op0=ALU.is_ge)
    e2 = small.tile([P, E], FP, tag="e2")
    nc.vector.tensor_mul(e2[:], ex[:], eq[:])
    nc.vector.tensor_sub(e2[:], ex[:], e2[:])
    m2 = small.tile([P, 1], FP, tag="m2")
    nc.vector.tensor_reduce(m2[:], e2[:], mybir.AxisListType.X, ALU.max)
    msk = small.tile([P, E], FP, tag="msk")
    nc.vector.tensor_scalar(out=msk[:], in0=ex[:], scalar1=m2[:],
                            scalar2=None, op0=ALU.is_ge)
    gp = small.tile([P, E], FP, tag="gp")
    nc.vector.tensor_mul(gp[:], ex[:], msk[:])
    dn = small.tile([P, 1], FP, tag="dn")
    nc.vector.tensor_reduce(dn[:], gp[:], mybir.AxisListType.X, ALU.add)
    rc = small.tile([P, 1], FP, tag="rc")
    nc.vector.reciprocal(rc[:], dn[:])
    gt = small.tile([P, E], FP, tag="gt")
    nc.scalar.activation(gt[:], gp[:], AF.Identity, scale=rc[:])
    nc.sync.dma_start(gates_shard[tt * P:(tt + 1) * P, :], gt[:])


def build_stage1(repeat=1, skip_collective=False, skip_attn=False):
    from contextlib import ExitStack

    nc = bacc.Bacc("TRN2", target_bir_lowering=False, debug=False,
                   num_devices=N_CORES)
    xT = nc.dram_tensor("xT", [D, S], FPR, kind="ExternalInput").ap()
    NWR = P + ND * 3 * P + D + P + 1
    NWF = P + ND * E
    env = dict(
        wpack_r=nc.dram_tensor("wpack_r", [P, NWR], FPR,
                               kind="ExternalInput").ap(),
        wpack_f=nc.dram_tensor("wpack_f", [P, NWF], FP,
                               kind="ExternalInput").ap(),
        corr=nc.dram_tensor("corr", [2, 3 * P], FPR,
                            kind="ExternalInput").ap(),
        srows_in=nc.dram_tensor("srows_in", [2, S], FPR,
                                kind="ExternalInput").ap(),
        rstd_in=nc.dram_tensor("rstd_in", [1, S], FP,
                               kind="ExternalInput").ap(),
        brr=nc.dram_tensor("brr", [1, E], FP, kind="ExternalInput").ap(),
        csw=nc.dram_tensor("csw", [1, E], FP, kind="ExternalInput").ap(),
    )
    xsh = nc.dram_tensor("xsh", [SSH, D], FP, kind="ExternalInput").ap()

    a_shard = nc.dram_tensor("a_shard", [SSH, D], FP,
                             kind="ExternalOutput").ap()
    h2_shard = nc.dram_tensor("h2_shard", [SSH, D], BF,
                              kind="ExternalOutput").ap()
    gates_shard = nc.dram_tensor("gates_shard", [SSH, E], FP,
                                 kind="ExternalOutput").ap()

    a_part = nc.dram_tensor("a_part", [S, D], FP)
    rs_out = nc.dram_tensor("rs_out", [SSH, D], FP)

    with tile.TileContext(nc) as tc, ExitStack() as st:
        pools = dict(
            xp=3, hp=2, stats=3, qkvT=1, vtile=1, expT=4, oT=2,
            misc=1, aout=2, small=3, post=2, srows=2,
        )
        for nm, bufs in pools.items():
            env[nm] = st.enter_context(tc.tile_pool(name=nm, bufs=bufs))
        for nm, bufs in dict(ps_rtr=1, ps_qkv=1, ps_mm=2, ps_acc=2).items():
            env[nm] = st.enter_context(
                tc.tile_pool(name=nm, bufs=bufs, space="PSUM"))

        _s1_setup(nc, env)

        from contextlib import nullcontext
        for _rep in ([0] if repeat > 1 else range(repeat)):
            loop_cm = tc.For_i(0, repeat, 1) if repeat > 1 else nullcontext()
            with loop_cm:
                # prefetch all x tiles
                xg_sb = []
                for g in range(NG):
                    xg = env["xp"].tile([P, ND, 512], FPR, tag="x")
                    nc.sync.dma_start(
                        xg[:], xT[:, g * 512:(g + 1) * 512].rearrange(
                            "(o p) s -> p o s", p=P))
                    xg_sb.append(xg)

                gen0 = _s1_qkv_gen(nc, env, 0, xg_sb)
                for _ in gen0:
                    pass
                for g in range(NG):
                    if skip_attn:
                        if g + 1 < NG:
                            for _ in _s1_qkv_gen(nc, env, g + 1, xg_sb):
                                pass
                        continue
                    nxt = (_s1_qkv_gen(nc, env, g + 1, xg_sb)
                           if g + 1 < NG else None)
                    _s1_group_attn(nc, env, g, a_part, filler=nxt)
                    if nxt is not None:
                        for _ in nxt:
                            pass
                    if not skip_collective:
                        nc.gpsimd.collective_compute(
                            "ReduceScatter", ALU.add,
                            replica_groups=[list(range(N_CORES))],
                            ins=[a_part[g * 512:(g + 1) * 512, :]],
                            outs=[rs_out[g * 64:(g + 1) * 64, :]],
                        )

                if skip_attn:
                    continue
                for tt in range(SSH // P):
                    _s1_post_tile(nc, env, tt, rs_out, xsh, a_shard, h2_shard,
                                  gates_shard)

    nc.compile()
    return nc


def build_stage2(repeat=1, C=512):
    nc = bacc.Bacc("TRN2", target_bir_lowering=False, debug=False,
                   num_devices=N_CORES)
    h2gT = nc.dram_tensor("h2gT", [P, ND * C], BF, kind="ExternalInput").ap()
    w1 = nc.dram_tensor("w1", [P, NF * ND * P], F83,
                        kind="ExternalInput").ap()
    b1 = nc.dram_tensor("b1", [P, NF], FP, kind="ExternalInput").ap()
    w2 = nc.dram_tensor("w2", [P, ND * NF * P], F83,
                        kind="ExternalInput").ap()
    b2 = nc.dram_tensor("b2", [P, ND], FP, kind="ExternalInput").ap()
    gates = nc.dram_tensor("gates", [1, C], FP, kind="ExternalInput").ap()
    outT = nc.dram_tensor("outT", [D, C], BF, kind="ExternalOutput").ap()

    c_splits = [(c0, min(512, C - c0)) for c0 in range(0, C, 512)]

    with tile.TileContext(nc) as tc:
        with (
            tc.tile_pool(name="wres", bufs=1) as wres_pool,
            tc.tile_pool(name="midT", bufs=1) as midT_pool,
            tc.tile_pool(name="misc", bufs=1) as misc_pool,
            tc.tile_pool(name="outp", bufs=1) as out_pool,
            tc.tile_pool(name="ps_a", bufs=2, space="PSUM") as ps_a,
            tc.tile_pool(name="ps_b", bufs=2, space="PSUM") as ps_b,
            tc.tile_pool(name="ps_c", bufs=2, space="PSUM") as ps_c,
            tc.tile_pool(name="ps_d", bufs=2, space="PSUM") as ps_d,
        ):
            h2gT_sb = wres_pool.tile([P, ND, C], BF)
            w1_sb = wres_pool.tile([P, NF, ND, P], F83)
            with tc.high_priority():
                for half in range(2):
                    nc.sync.dma_start(
                        h2gT_sb[:, half * 4:(half + 1) * 4],
                        h2gT[:, half * 4 * C:(half + 1) * 4 * C].rearrange(
                            "p (o c) -> p o c", o=4))
                for q in range(2):
                    fq = NF // 8
                    nc.sync.dma_start(
                        w1_sb[:, q * fq:(q + 1) * fq],
                        w1[:, q * fq * ND * P:(q + 1) * fq * ND * P]
                        .rearrange("p (f o m) -> p f o m", f=fq, o=ND))
            for q in range(2, 8):
                fq = NF // 8
                nc.sync.dma_start(
                    w1_sb[:, q * fq:(q + 1) * fq],
                    w1[:, q * fq * ND * P:(q + 1) * fq * ND * P].rearrange(
                        "p (f o m) -> p f o m", f=fq, o=ND))
            b1_sb = misc_pool.tile([P, NF], FP)   # b1_sb[p, ft] = b1[ft*128+p]
            b2_sb = misc_pool.tile([P, ND], FP)   # b2_sb[p, dt] = b2[dt*128+p]
            gates_row = misc_pool.tile([1, C], FP)
            gates_bc = misc_pool.tile([P, C], FP)
            with tc.high_priority():
                nc.sync.dma_start(b1_sb[:], b1[:])
                nc.sync.dma_start(b2_sb[:], b2[:])
                nc.sync.dma_start(gates_row[:], gates[:])
                nc.gpsimd.partition_broadcast(gates_bc[:], gates_row[:])
            w2_sb = wres_pool.tile([P, ND, NF, P], F83)

            from contextlib import nullcontext
            for _rep in ([0] if repeat > 1 else range(repeat)):
              loop_cm = tc.For_i(0, repeat, 1) if repeat > 1 else nullcontext()
              with loop_cm:
                midT_sb = []
                for ft in range(NF):
                    mt = midT_pool.tile([P, C], BF, tag=f"midT{ft}")
                    midT_sb.append(mt)
                o_sb = []
                for dt in range(ND):
                    ot = out_pool.tile([P, C], BF, tag=f"osb{dt}")
                    o_sb.append(ot)
                for (c0, cn) in c_splits:
                    cs = slice(c0, c0 + cn)
                    # phase 1: midT[f, cs] = gelu(w1.T @ h2gT + b1)
                    for ft in range(NF):
                        if c0 == 0 and ft % 4 == 0 and _rep == 0:
                            dt = ft // 4
                            with tc.high_priority(offset=-10000):
                                nc.gpsimd.dma_start(
                                    w2_sb[:, dt],
                                    w2[:, dt * NF * P:(dt + 1) * NF * P]
                                    .rearrange("p (f m) -> p f m", f=NF))
                        mid_ps = (ps_a if cn > 128 else ps_b).tile(
                            [P, cn], FP, tag="mid")
                        for d in range(ND):
                            nc.tensor.matmul(
                                mid_ps[:], w1_sb[:, ft, d, :],
                                h2gT_sb[:, d, cs],
                                start=(d == 0), stop=(d == ND - 1))
                        nc.scalar.activation(
                            midT_sb[ft][:, cs], mid_ps[:], AF.Gelu,
                            bias=b1_sb[:, ft:ft + 1], scale=1.0 / W1S)
                    # phase 2: outT[dcol, cs] = (w2.T @ midT + b2) * gates
                    for dt in range(ND):
                        o_ps = (ps_c if cn > 128 else ps_d).tile(
                            [P, cn], FP, tag="out")
                        for ft in range(NF):
                            nc.tensor.matmul(
                                o_ps[:], w2_sb[:, dt, ft, :],
                                midT_sb[ft][:, cs],
                                start=(ft == 0), stop=(ft == NF - 1))
                        nc.vector.scalar_tensor_tensor(
                            out=o_sb[dt][:, cs], in0=o_ps[:],
                            scalar=b2_sb[:, dt:dt + 1],
                            in1=gates_bc[:, cs],
                            op0=ALU.add, op1=ALU.mult)
                        nc.sync.dma_start(
                            outT[dt * P:(dt + 1) * P, cs], o_sb[dt][:, cs])

    nc.compile()
    return nc


_CACHE = {}


def _get_stage(name, repeat=1, **kw):
    key = (name, repeat, tuple(sorted(kw.items())))
    if key not in _CACHE:
        nc = (build_stage1(repeat, **kw) if name == "s1"
              else build_stage2(repeat, **kw))
        _CACHE[key] = _make_runner(nc)
    return _CACHE[key]


def _make_runner(nc):
    """Build a reusable sharded jitted callable for an SPMD bass program."""
    import jax
    from jax.sharding import Mesh, PartitionSpec
    from jax.experimental.shard_map import shard_map
    import concourse.bass2jax as bass2jax

    bass2jax.install_neuronx_cc_hook()
    partition_name = (nc.partition_id_tensor.name
                      if nc.partition_id_tensor else None)
    in_names, out_names, out_avals, zero_outs = [], [], [], []
    for alloc in nc.m.functions[0].allocations:
        if not isinstance(alloc, mybir.MemoryLocationSet):
            continue
        name = alloc.memorylocations[0].name
        if alloc.kind == "ExternalInput":
            if name != partition_name:
                in_names.append(name)
        elif alloc.kind == "ExternalOutput":
            out_names.append(name)
            shape = tuple(alloc.tensor_shape)
            dtype = mybir.dt.np(alloc.dtype)
            out_avals.append(jax.core.ShapedArray(shape, dtype))
            zero_outs.append(np.zeros(shape, dtype))
    n_params = len(in_names)
    n_outs = len(out_avals)
    in_names_all = in_names + out_names
    if partition_name is not None:
        in_names_all = in_names_all + [partition_name]

    def _body(*args):
        operands = list(args)
        if partition_name is not None:
            operands.append(bass2jax.partition_id_tensor())
        outs = bass2jax._bass_exec_p.bind(
            *operands,
            out_avals=tuple(out_avals),
            in_names=tuple(in_names_all),
            out_names=tuple(out_names),
            lowering_input_output_aliases=(),
            sim_require_finite=True,
            sim_require_nnan=True,
            nc=nc,
        )
        return tuple(outs)

    devices = jax.devices()[:N_CORES]
    mesh = Mesh(np.asarray(devices), ("core",))
    in_specs = (PartitionSpec("core"),) * (n_params + n_outs)
    out_specs = (PartitionSpec("core"),) * len(out_names)
    sharded = jax.jit(
        shard_map(_body, mesh=mesh, in_specs=in_specs, out_specs=out_specs,
                  check_rep=False),
        keep_unused=True,
    )

    class Runner:
        pass

    r = Runner()
    r.nc = nc
    r.sharded = sharded
    r.in_names = in_names
    r.out_names = out_names
    r.zero_outs = zero_outs
    r.out_avals = out_avals
    return r


def _run_spmd(runner, in_maps):
    concat_in = [
        np.concatenate([np.asarray(in_maps[c][nm]) for c in range(N_CORES)],
                       axis=0)
        for nm in runner.in_names
    ]
    concat_zeros = [
        np.zeros((N_CORES * z.shape[0], *z.shape[1:]), z.dtype)
        for z in runner.zero_outs
    ]
    outs = runner.sharded(*concat_in, *concat_zeros)
    return [
        {nm: np.asarray(outs[i]).reshape(N_CORES,
                                         *runner.out_avals[i].shape)[c]
         for i, nm in enumerate(runner.out_names)}
        for c in range(N_CORES)
    ]


def _token_map():
    """TOK[c, p] = global token index held by core c at shard position p,
    under the per-512-group ReduceScatter layout."""
    TOK = np.empty((N_CORES, SSH), np.int64)
    for c in range(N_CORES):
        p = np.arange(SSH)
        g = p // 64
        TOK[c] = 512 * g + c * 64 + (p % 64)
    return TOK


_TOK = _token_map()


def _stage1_in_maps(inputs):
    x = np.ascontiguousarray(np.asarray(inputs["x"], np.float32)[0])
    g1 = np.asarray(inputs["ln1_g"], np.float32)
    b1v = np.asarray(inputs["ln1_b"], np.float32)
    g2 = np.asarray(inputs["ln2_g"], np.float32)
    b2v = np.asarray(inputs["ln2_b"], np.float32)
    Wq, bq = (np.asarray(inputs["Wq"], np.float32),
              np.asarray(inputs["bq"], np.float32))
    Wk, bk = (np.asarray(inputs["Wk"], np.float32),
              np.asarray(inputs["bk"], np.float32))
    Wv, bv = (np.asarray(inputs["Wv"], np.float32),
              np.asarray(inputs["bv"], np.float32))
    Wo, bo = (np.asarray(inputs["Wo"], np.float32),
              np.asarray(inputs["bo"], np.float32))
    Wr, br = (np.asarray(inputs["Wr"], np.float32),
              np.asarray(inputs["br"], np.float32))

    Wqf, bqf = g1[:, None] * Wq, bq + b1v @ Wq
    Wkf, bkf = g1[:, None] * Wk, bk + b1v @ Wk
    Wvf, bvf = g1[:, None] * Wv, bv + b1v @ Wv
    Wrf, brf = g2[:, None] * Wr, br + b2v @ Wr

    tri = np.triu(np.ones((P, P), np.float32))

    m = x.mean(axis=1)
    var = x.var(axis=1)
    std = np.sqrt(var + EPS)
    common = dict(
        xT=np.ascontiguousarray(x.T.astype(np.float32)),
        srows_in=np.ascontiguousarray(
            np.stack([std, -m]).astype(np.float32)),
        rstd_in=np.ascontiguousarray((1.0 / std).astype(np.float32))[None, :],
        brr=brf.astype(np.float32)[None, :],
        csw=Wrf.sum(axis=0).astype(np.float32)[None, :],
    )
    wr_cols = np.concatenate(
        [Wrf[d * P:(d + 1) * P, :] for d in range(ND)], axis=1)
    common["wpack_f"] = np.ascontiguousarray(np.concatenate(
        [np.eye(P, dtype=np.float32), wr_cols], axis=1).astype(np.float32))
    in_maps = []
    for c in range(N_CORES):
        cols = slice(c * HPC * HD, (c + 1) * HPC * HD)
        wqkv = np.concatenate([Wqf[:, cols], Wkf[:, cols], Wvf[:, cols]],
                              axis=1)
        bqkv = np.concatenate([bqf[cols], bkf[cols], bvf[cols]])
        wpack_r = np.concatenate(
            [np.eye(P, dtype=np.float32)] +
            [wqkv[d * P:(d + 1) * P, :] for d in range(ND)] +
            [Wo[cols, :], tri,
             np.full((P, 1), 1.0 / D, np.float32)], axis=1)
        corr = np.stack([bqkv, wqkv.sum(axis=0)])
        m = dict(common)
        m.update(
            wpack_r=np.ascontiguousarray(wpack_r.astype(np.float32)),
            corr=np.ascontiguousarray(corr.astype(np.float32)),
            xsh=np.ascontiguousarray((x[_TOK[c]] + bo).astype(np.float32)),
        )
        in_maps.append(m)
    return in_maps


def _pick_capacity(max_load):
    # device capacity is fixed at 512 (single PSUM column-split, minimal
    # instruction count); tokens beyond 512 per expert are computed on host.
    return 512


def _stage2_in_maps(h2, gates, inputs, C):
    g2 = np.asarray(inputs["ln2_g"], np.float32)
    b2v = np.asarray(inputs["ln2_b"], np.float32)
    e_w1 = np.asarray(inputs["e_w1"], np.float32)
    e_b1 = np.asarray(inputs["e_b1"], np.float32)
    e_w2 = np.asarray(inputs["e_w2"], np.float32)
    e_b2 = np.asarray(inputs["e_b2"], np.float32)

    in_maps, idxs = [], []
    for e in range(N_CORES):
        idx_full = np.nonzero(gates[:, e] > 0.0)[0]
        idxs.append(idx_full)
        idx = idx_full[:C]
        h2g = np.zeros((C, D), np.float32)
        h2g[:len(idx)] = h2[idx]
        gv = np.zeros((C,), np.float32)
        gv[:len(idx)] = gates[idx, e]
        w1f = (g2[:, None] * e_w1[e]).astype(np.float32)
        b1f = e_b1[e] + b2v @ e_w1[e]
        w1host = np.ascontiguousarray(
            (w1f * W1S).reshape(ND, P, NF, P).transpose(1, 2, 0, 3).reshape(
                P, NF * ND * P)).astype(NP_F83)
        w2host = np.ascontiguousarray(
            (e_w2[e] * W2S).reshape(NF, P, ND, P).transpose(1, 2, 0, 3)
            .reshape(P, ND * NF * P)).astype(NP_F83)
        h2gT_host = np.ascontiguousarray(
            h2g.T.reshape(ND, P, C).transpose(1, 0, 2).reshape(
                P, ND * C)).astype(NP_BF16)
        in_maps.append(dict(
            h2gT=h2gT_host,
            w1=w1host,
            b1=np.ascontiguousarray(b1f.reshape(NF, P).T.astype(np.float32)),
            w2=w2host,
            b2=np.ascontiguousarray(
                (e_b2[e] * W2S).reshape(ND, P).T.astype(np.float32)),
            gates=(gv / W2S)[None, :],
        ))
    return in_maps, idxs


def kernel(**inputs):
    r1 = _get_stage("s1")
    in_maps1 = _stage1_in_maps(inputs)
    res1 = _run_spmd(r1, in_maps1)

    order = _TOK.reshape(-1)
    a = np.empty((S, D), np.float32)
    a[order] = np.concatenate([res1[c]["a_shard"] for c in range(N_CORES)])
    h2 = np.empty((S, D), np.float32)
    h2[order] = np.concatenate(
        [np.asarray(res1[c]["h2_shard"], np.float32) for c in range(N_CORES)])
    gates = np.empty((S, E), np.float32)
    gates[order] = np.concatenate(
        [res1[c]["gates_shard"] for c in range(N_CORES)])

    loads = (gates > 0.0).sum(axis=0)
    C = _pick_capacity(loads.max())
    r2 = _get_stage("s2", C=C)
    in_maps2, idxs = _stage2_in_maps(h2, gates, inputs, C)
    res2 = _run_spmd(r2, in_maps2)

    out = a
    g2 = np.asarray(inputs["ln2_g"], np.float32)
    b2v = np.asarray(inputs["ln2_b"], np.float32)
    for e in range(N_CORES):
        idx = idxs[e]
        ndev = min(len(idx), C)
        out[idx[:ndev]] += np.asarray(res2[e]["outT"][:, :ndev],
                                      np.float32).T
        if len(idx) > ndev:
            # host tail: exact fp32 FFN for the few overflow tokens
            tidx = idx[ndev:]
            w1f = g2[:, None] * np.asarray(inputs["e_w1"], np.float32)[e]
            b1f = (np.asarray(inputs["e_b1"], np.float32)[e]
                   + b2v @ np.asarray(inputs["e_w1"], np.float32)[e])
            pre = h2[tidx] @ w1f + b1f
            try:
                from scipy.special import erf
            except ImportError:
                import math
                _erf = np.frompyfunc(math.erf, 1, 1)

                def erf(z):
                    return _erf(z).astype(np.float32)
            mid = pre * 0.5 * (1.0 + erf(pre / np.sqrt(2.0)))
            eo = (mid @ np.asarray(inputs["e_w2"], np.float32)[e]
                  + np.asarray(inputs["e_b2"], np.float32)[e])
            out[tidx] += eo * gates[tidx, e][:, None]
    return out.reshape(1, S, D).astype(np.float32)



# revision 27
# speedup vs baseline: 1.1258x; 1.1258x over previous
"""Trainium2 Bass kernel for nn_Block_73443940761664 (moe_routing).

Transformer block: LN1 -> causal MHA -> residual -> LN2 -> top-2-of-8
sparse MoE (dense-equivalent combine) -> residual.

Distribution over 8 NeuronCores:
  dispatch 1: attention head-parallel (2 heads/core), all in fp32/fp32r
              (full PE rate at moving dim >= 256).  LN1 and the per-token
              1/std are folded into the weights and the host-prescaled xT,
              so the device computes only matmuls + one-pass exp-softmax
              (ones-column denominator trick).  Per-512-token-group
              ReduceScatter (fp32) of the Wo partial sums overlaps the
              next group's compute; next-group QKV work is interleaved
              into the attention kb loop via a generator filler, and the
              kb loop is issued with a 2-step scores/exp lookahead.
  host:       residual add, LN2, fp32 router, exact top-2 softmax gates
              (margins down to 2.3e-5 survive), capacity-512 per-expert
              token gather; all pure glue feeding dispatch 2.
  dispatch 2: expert-parallel FFN (1 expert/core) on gathered tokens,
              both phases fp8e4m3 DoubleRow (2x PE rate, 256-deep
              contraction per matmul); gelu applies bias + 1/W1S descale;
              gates carry 1/W2S; host scatter-adds into the output and
              computes the few above-capacity overflow tokens exactly.
"""

import numpy as np
import ml_dtypes

import concourse.tile as tile
import concourse.mybir as mybir
from concourse import bacc

P = 128
S = 2048
D = 1024
HD = 64           # head dim
HPC = 2           # heads per core
E = 8
F = 4096
N_CORES = 8
FP = mybir.dt.float32
FPR = mybir.dt.float32r
BF = mybir.dt.bfloat16
F8 = mybir.dt.float8e4    # e4m3 (DoubleRow capable)
F83 = mybir.dt.float8e3   # e3m4 (more mantissa, bf16-speed matmul)
AF = mybir.ActivationFunctionType
ALU = mybir.AluOpType
DR = mybir.MatmulPerfMode.DoubleRow
EPS = 1e-5

NT = S // P       # 16 token tiles
NG = S // 512     # 4 token groups of 512
ND = D // P       # 8 d-chunks
NF = F // P       # 32 f-tiles
SSH = S // N_CORES  # 256 tokens per core shard

NP_BF16 = ml_dtypes.bfloat16
W1S = 64.0   # host premultiplies W1; device applies 1/W1S in the gelu
W2S = 64.0   # host premultiplies W2; gates carry 1/W2S
NP_F8 = ml_dtypes.float8_e4m3
NP_F83 = ml_dtypes.float8_e3m4
NP_W1 = NP_F8    # stage-2 weight/activation wire dtypes (DoubleRow: e4m3)
NP_W2 = NP_F8
NP_H2G = NP_F8


def _s1_setup(nc, env):
    """Load weights/constants into SBUF; populate env."""
    misc_pool = env["misc"]
    # packed fp32r constants: [idenr | wqkv d0..d7 | wo | tri | onesd]
    NWR = P + ND * 3 * P + D + P + 1
    wrk = misc_pool.tile([P, NWR], FPR)
    nc.scalar.dma_start(wrk[:], env["wpack_r"][:])
    idenr_sb = wrk[:, 0:P]
    wqkv_sb = [wrk[:, P + d * 3 * P:P + (d + 1) * 3 * P] for d in range(ND)]
    o0 = P + ND * 3 * P
    wo_sb = wrk[:, o0:o0 + D]
    tri_sb = wrk[:, o0 + D:o0 + D + P]
    corr2_sb = misc_pool.tile([2, 3 * P], FPR)
    nc.sync.dma_start(corr2_sb[:], env["corr"][:])
    srows_sb = misc_pool.tile([2, S], FPR)
    nc.sync.dma_start(srows_sb[:], env["srows_in"][:])

    qT_sb = env["qkvT"].tile([P, S], FPR)   # rows: h0 0:64 | h1 64:128
    kT_sb = env["qkvT"].tile([P, S], FPR)
    vT_sb = env["qkvT"].tile([P, S], FPR)
    # v_sb[kb]: [tok, 130] = [h0 v 0:64 | ones 64 | h1 v 65:129 | ones 129]
    ones_f = misc_pool.tile([P, 1], FP)
    nc.vector.memset(ones_f[:], 1.0)
    v_sb = []
    for kb in range(NT):
        vkb = env["vtile"].tile([P, 2 * (HD + 1)], FPR, tag=f"v{kb}")
        for h in range(HPC):
            nc.scalar.activation(vkb[:, h * (HD + 1) + HD:
                                     h * (HD + 1) + HD + 1],
                                 ones_f[:], AF.Identity)
        v_sb.append(vkb)
    env.update(idenr_sb=idenr_sb,
               wqkv_sb=wqkv_sb, wo_sb=wo_sb, tri_sb=tri_sb,
               corr2_sb=corr2_sb,
               srows_sb=srows_sb,
               qT_sb=qT_sb, kT_sb=kT_sb, vT_sb=vT_sb, v_sb=v_sb)


def _s1_qkv_gen(nc, env, g, xg_sb):
    """Generator emitting group g's QKV work in small chunks.

    Yields between chunks so the caller can interleave these PE/DVE/Pool
    instructions into attention(g-1)'s kb loop, filling exp-latency bubbles.
    """
    xT_g = xg_sb[g]
    col = slice(g * 512, (g + 1) * 512)
    srow_g = env["srows_sb"][:, col]          # [2, 512]: [1; -m*rstd]
    yield
    for which, dst in ((0, env["qT_sb"]), (1, env["kT_sb"]),
                       (2, env["vT_sb"])):
        ps = env["ps_qkv"].tile([P, 512], FP, tag=f"qkv{which}")
        wcol = slice(which * P, (which + 1) * P)
        for d in range(ND):
            nc.tensor.matmul(
                ps[:], env["wqkv_sb"][d][:, wcol], xT_g[:, d, :],
                start=(d == 0), stop=False)
            if d % 2 == 1:
                yield
        nc.tensor.matmul(ps[:], env["corr2_sb"][:, wcol], srow_g,
                         start=False, stop=True)
        nc.vector.tensor_copy(dst[:, col], ps[:])
        yield
    # v transposed to [tok, dim] per key tile via PE
    for ti in range(4):
        kb = g * 4 + ti
        for h in range(HPC):
            ps = env["ps_rtr"].tile([P, HD], FPR, tag="rtr")
            nc.tensor.transpose(
                ps[:], env["vT_sb"][h * HD:(h + 1) * HD,
                                    kb * P:(kb + 1) * P],
                env["idenr_sb"][h * HD:(h + 1) * HD, h * HD:(h + 1) * HD])
            base = h * (HD + 1)
            nc.vector.tensor_copy(env["v_sb"][kb][:, base:base + HD], ps[:])
            yield


def _s1_group_attn(nc, env, g, a_part, filler=None,
                   skip_exp=False, skip_wo=False):
    """Causal attention + Wo partial for group g; write a_part tiles.
    ``filler()`` emits one chunk of next-group QKV work per kb step.

    The kb loop is software-pipelined with lookahead L: scores/exp for
    step i are issued L steps before the AV matmul that consumes them, so
    the in-order PE queue never stalls on the Act engine's exp latency.
    skip_exp/skip_wo are timing-attribution ablations (wrong results)."""
    L = 2
    def fill():
        if filler is not None:
            next(filler, None)
    qcol = slice(g * 512, (g + 1) * 512)
    oT_sb = env["oT"].tile([P, 512], FPR, tag="oT")
    for h in range(HPC):
        acc = env["ps_acc"].tile([HD + 1, 512], FP, tag="acc")
        nkb = g * 4 + 4
        base = h * (HD + 1)
        # diagonal tiles first (their extra mask hop overlaps later work);
        # the first diagonal tile covers the full 512 columns, so the
        # accumulation region is fully initialized by the start flag.
        order = list(range(g * 4, nkb)) + list(range(0, g * 4))
        pending = []
        for i in range(nkb + L):
            if i < nkb:
                kb = order[i]
                j = kb - g * 4          # >= 0 on the diagonal group
                c0 = max(0, j) * P      # first unmasked query column
                cs = slice(c0, 512)
                qs = slice(g * 512 + c0, (g + 1) * 512)
                sc = env["ps_mm"].tile([P, 512], FP, tag="mm")
                nc.tensor.matmul(
                    sc[:, cs], env["kT_sb"][h * HD:(h + 1) * HD,
                                            kb * P:(kb + 1) * P],
                    env["qT_sb"][h * HD:(h + 1) * HD, qs],
                    start=True, stop=True)
                if skip_exp:
                    # timing-only stand-in moving operand (content garbage)
                    et = env["qT_sb"][:, 0:512]
                else:
                    et = env["expT"].tile([P, 512], FPR, tag="exp")
                    nc.scalar.activation(et[:, cs], sc[:, cs], AF.Exp,
                                         scale=0.125)
                    if j >= 0:
                        # triangular mask only on the 128-wide diag block
                        nc.vector.tensor_mul(et[:, c0:c0 + P],
                                             et[:, c0:c0 + P],
                                             env["tri_sb"][:])
                pending.append((kb, cs, et))
            if i >= L:
                ii = i - L
                kb, cs, et = pending[ii]
                nc.tensor.matmul(
                    acc[:, cs], env["v_sb"][kb][:, base:base + HD + 1],
                    et[:, cs],
                    start=(ii == 0), stop=(ii == nkb - 1))
            fill()
        if skip_wo:
            continue
        rec = env["small"].tile([1, 512], FPR, tag="rec")
        with nc.allow_low_precision(reason="fp32r shares fp32 bits"):
            nc.vector.reciprocal(rec[:], acc[HD:HD + 1, :])
        recb = env["ps_rtr"].tile([HD, 512], FP, tag="rtr")
        # tri row 0 is all-ones fp32r: [1, HD] broadcast stationary
        nc.tensor.matmul(recb[:], env["tri_sb"][0:1, 0:HD], rec[:],
                         start=True, stop=True)
        recb_sb = env["small"].tile([HD, 512], FP, tag="recsb")
        nc.vector.tensor_copy(recb_sb[:], recb[:])
        nc.vector.tensor_mul(oT_sb[h * HD:(h + 1) * HD, :],
                             acc[0:HD, :], recb_sb[:])
    if skip_wo:
        return
    # drain all remaining filler (next group's QKV) BEFORE the Wo matmuls:
    # gives the PE queue independent work while the normalize chains
    # (recip -> PE bcast -> copy -> mul, per head) complete, so the Wo
    # matmuls don't head-of-line block the engine at the group boundary.
    if filler is not None:
        for _ in filler:
            pass
    for ti in range(4):
        t_idx = g * 4 + ti
        asb = env["aout"].tile([P, D], FP, tag="a")
        for half in range(2):
            colh = slice(half * 512, (half + 1) * 512)
            ps = env["ps_mm"].tile([P, 512], FP, tag="mm")
            nc.tensor.matmul(ps[:], oT_sb[:, ti * P:(ti + 1) * P],
                             env["wo_sb"][:, colh], start=True, stop=True)
            if half == 0:
                nc.scalar.activation(asb[:, colh], ps[:], AF.Identity)
            else:
                nc.vector.tensor_copy(asb[:, colh], ps[:])
        nc.sync.dma_start(a_part[t_idx * P:(t_idx + 1) * P, :], asb[:])


def build_stage1(repeat=1, skip_collective=False, skip_attn=False,
                 skip_exp=False, skip_wo=False):
    """LN1-folded QKV + causal MHA + Wo partials + per-group ReduceScatter.

    Residual add, LN2, router and top-2 gates all moved to the host (they
    feed only the host-side expert gather, not further device compute), so
    the device program ends at the last RS chunk."""
    from contextlib import ExitStack

    nc = bacc.Bacc("TRN2", target_bir_lowering=False, debug=False,
                   num_devices=N_CORES)
    xT = nc.dram_tensor("xT", [D, S], FPR, kind="ExternalInput").ap()
    NWR = P + ND * 3 * P + D + P + 1
    env = dict(
        wpack_r=nc.dram_tensor("wpack_r", [P, NWR], FPR,
                               kind="ExternalInput").ap(),
        corr=nc.dram_tensor("corr", [2, 3 * P], FPR,
                            kind="ExternalInput").ap(),
        srows_in=nc.dram_tensor("srows_in", [2, S], FPR,
                                kind="ExternalInput").ap(),
    )
    rs_ext = nc.dram_tensor("rs_ext", [SSH, D], FP,
                            kind="ExternalOutput").ap()
    rs_out = nc.dram_tensor("rs_out", [SSH, D], FP)
    a_part = nc.dram_tensor("a_part", [S, D], FP)

    with tile.TileContext(nc) as tc, ExitStack() as st:
        pools = dict(
            xp=4, qkvT=1, vtile=1, expT=6, oT=2,
            misc=1, aout=4, small=3, srows=2,
        )
        for nm, bufs in pools.items():
            env[nm] = st.enter_context(tc.tile_pool(name=nm, bufs=bufs))
        for nm, bufs in dict(ps_rtr=1, ps_qkv=1, ps_mm=3, ps_acc=1).items():
            env[nm] = st.enter_context(
                tc.tile_pool(name=nm, bufs=bufs, space="PSUM"))

        _s1_setup(nc, env)

        from contextlib import nullcontext
        for _rep in ([0] if repeat > 1 else range(repeat)):
            loop_cm = tc.For_i(0, repeat, 1) if repeat > 1 else nullcontext()
            with loop_cm:
                # prefetch all x tiles
                xg_sb = []
                for g in range(NG):
                    xg = env["xp"].tile([P, ND, 512], FPR, tag="x")
                    nc.sync.dma_start(
                        xg[:], xT[:, g * 512:(g + 1) * 512].rearrange(
                            "(o p) s -> p o s", p=P))
                    xg_sb.append(xg)

                gen0 = _s1_qkv_gen(nc, env, 0, xg_sb)
                for _ in gen0:
                    pass
                for g in range(NG):
                    if skip_attn:
                        if g + 1 < NG:
                            for _ in _s1_qkv_gen(nc, env, g + 1, xg_sb):
                                pass
                        continue
                    nxt = (_s1_qkv_gen(nc, env, g + 1, xg_sb)
                           if g + 1 < NG else None)
                    _s1_group_attn(nc, env, g, a_part, filler=nxt,
                                   skip_exp=skip_exp, skip_wo=skip_wo)
                    if nxt is not None:
                        for _ in nxt:
                            pass
                    if not skip_collective and not skip_wo:
                        nc.gpsimd.collective_compute(
                            "ReduceScatter", ALU.add,
                            replica_groups=[list(range(N_CORES))],
                            ins=[a_part[g * 512:(g + 1) * 512, :]],
                            outs=[rs_out[g * 64:(g + 1) * 64, :]],
                        )
                        # collectives cannot write IO tensors; bounce via DMA
                        nc.sync.dma_start(rs_ext[g * 64:(g + 1) * 64, :],
                                          rs_out[g * 64:(g + 1) * 64, :])

    nc.compile()
    return nc


def build_stage2(repeat=1, C=512):
    """Expert FFN, both phases fp8e4m3 DoubleRow (0.5 cyc/row on PE).

    Phase 1: mid[ft] = gelu(sum_t w1[:,ft,2t:2t+2].T @ h2[2t:2t+2] + b1),
    4 DR matmuls of 256-deep contraction each.  midT stored as 16 tiles of
    [P, 2, C] e4m3 so phase 2's moving operand is pair-shaped in place.
    Phase 2: out[dt] = (sum_t w2[:,dt,2t:2t+2].T @ midT[t] + b2) * gates,
    16 DR matmuls.
    """
    assert C == 512
    NPD = ND // 2    # 4 d-pairs
    NPF = NF // 2    # 16 f-pairs
    nc = bacc.Bacc("TRN2", target_bir_lowering=False, debug=False,
                   num_devices=N_CORES)
    h2gT = nc.dram_tensor("h2gT", [P, ND * C], F8, kind="ExternalInput").ap()
    w1 = nc.dram_tensor("w1", [P, NF * ND * P], F8,
                        kind="ExternalInput").ap()
    b1 = nc.dram_tensor("b1", [P, NF], FP, kind="ExternalInput").ap()
    w2 = nc.dram_tensor("w2", [P, ND * NF * P], F8,
                        kind="ExternalInput").ap()
    b2 = nc.dram_tensor("b2", [P, ND], FP, kind="ExternalInput").ap()
    gates = nc.dram_tensor("gates", [1, C], FP, kind="ExternalInput").ap()
    outT = nc.dram_tensor("outT", [D, C], BF, kind="ExternalOutput").ap()

    with tile.TileContext(nc) as tc:
        with (
            tc.tile_pool(name="wres", bufs=1) as wres_pool,
            tc.tile_pool(name="midT", bufs=1) as midT_pool,
            tc.tile_pool(name="misc", bufs=1) as misc_pool,
            tc.tile_pool(name="outp", bufs=1) as out_pool,
            tc.tile_pool(name="ps_a", bufs=3, space="PSUM") as ps_a,
            tc.tile_pool(name="ps_b", bufs=3, space="PSUM") as ps_b,
        ):
            h2gT_sb = wres_pool.tile([P, ND, C], F8)
            w1_sb = wres_pool.tile([P, NF, ND, P], F8)
            with tc.high_priority():
                for half in range(2):
                    nc.sync.dma_start(
                        h2gT_sb[:, half * 4:(half + 1) * 4],
                        h2gT[:, half * 4 * C:(half + 1) * 4 * C].rearrange(
                            "p (o c) -> p o c", o=4))
                for q in range(2):
                    fq = NF // 8
                    nc.sync.dma_start(
                        w1_sb[:, q * fq:(q + 1) * fq],
                        w1[:, q * fq * ND * P:(q + 1) * fq * ND * P]
                        .rearrange("p (f o m) -> p f o m", f=fq, o=ND))
            for q in range(2, 8):
                fq = NF // 8
                nc.sync.dma_start(
                    w1_sb[:, q * fq:(q + 1) * fq],
                    w1[:, q * fq * ND * P:(q + 1) * fq * ND * P].rearrange(
                        "p (f o m) -> p f o m", f=fq, o=ND))
            b1_sb = misc_pool.tile([P, NF], FP)   # b1_sb[p, ft] = b1[ft*128+p]
            b2_sb = misc_pool.tile([P, ND], FP)   # b2_sb[p, dt] = b2[dt*128+p]
            gates_row = misc_pool.tile([1, C], FP)
            gates_bc = misc_pool.tile([P, C], FP)
            with tc.high_priority():
                nc.sync.dma_start(b1_sb[:], b1[:])
                nc.sync.dma_start(b2_sb[:], b2[:])
                nc.sync.dma_start(gates_row[:], gates[:])
                nc.gpsimd.partition_broadcast(gates_bc[:], gates_row[:])
            w2_sb = wres_pool.tile([P, ND, NF, P], F8)

            from contextlib import nullcontext
            for _rep in ([0] if repeat > 1 else range(repeat)):
              loop_cm = tc.For_i(0, repeat, 1) if repeat > 1 else nullcontext()
              with loop_cm:
                midT_sb = []
                for fp in range(NPF):
                    mt = midT_pool.tile([P, 2, C], F8, tag=f"midT{fp}")
                    midT_sb.append(mt)
                o_sb = []
                for dt in range(ND):
                    ot = out_pool.tile([P, C], BF, tag=f"osb{dt}")
                    o_sb.append(ot)
                # phase 1: midT[f] = gelu(w1.T @ h2gT + b1)
                for ft in range(NF):
                    if ft % 4 == 0 and _rep == 0:
                        dt = ft // 4
                        with tc.high_priority(offset=-10000):
                            nc.gpsimd.dma_start(
                                w2_sb[:, dt],
                                w2[:, dt * NF * P:(dt + 1) * NF * P]
                                .rearrange("p (f m) -> p f m", f=NF))
                    mid_ps = ps_a.tile([P, C], FP, tag="mid")
                    for t in range(NPD):
                        nc.tensor.matmul(
                            mid_ps[:], w1_sb[:, ft, 2 * t:2 * t + 2, :],
                            h2gT_sb[:, 2 * t:2 * t + 2, :],
                            start=(t == 0), stop=(t == NPD - 1),
                            perf_mode=DR)
                    nc.scalar.activation(
                        midT_sb[ft // 2][:, ft % 2, :], mid_ps[:], AF.Gelu,
                        bias=b1_sb[:, ft:ft + 1], scale=1.0 / W1S)
                # phase 2: outT[dcol] = (w2.T @ midT + b2) * gates
                for dt in range(ND):
                    o_ps = ps_b.tile([P, C], FP, tag="out")
                    for t in range(NPF):
                        nc.tensor.matmul(
                            o_ps[:], w2_sb[:, dt, 2 * t:2 * t + 2, :],
                            midT_sb[t][:],
                            start=(t == 0), stop=(t == NPF - 1),
                            perf_mode=DR)
                    nc.vector.scalar_tensor_tensor(
                        out=o_sb[dt][:], in0=o_ps[:],
                        scalar=b2_sb[:, dt:dt + 1],
                        in1=gates_bc[:],
                        op0=ALU.add, op1=ALU.mult)
                    nc.sync.dma_start(
                        outT[dt * P:(dt + 1) * P, :], o_sb[dt][:])

    nc.compile()
    return nc


_CACHE = {}


def _get_stage(name, repeat=1, **kw):
    key = (name, repeat, tuple(sorted(kw.items())))
    if key not in _CACHE:
        nc = (build_stage1(repeat, **kw) if name == "s1"
              else build_stage2(repeat, **kw))
        _CACHE[key] = _make_runner(nc)
    return _CACHE[key]


def _make_runner(nc):
    """Build a reusable sharded jitted callable for an SPMD bass program."""
    import jax
    from jax.sharding import Mesh, PartitionSpec
    from jax.experimental.shard_map import shard_map
    import concourse.bass2jax as bass2jax

    bass2jax.install_neuronx_cc_hook()
    partition_name = (nc.partition_id_tensor.name
                      if nc.partition_id_tensor else None)
    in_names, out_names, out_avals, zero_outs = [], [], [], []
    for alloc in nc.m.functions[0].allocations:
        if not isinstance(alloc, mybir.MemoryLocationSet):
            continue
        name = alloc.memorylocations[0].name
        if alloc.kind == "ExternalInput":
            if name != partition_name:
                in_names.append(name)
        elif alloc.kind == "ExternalOutput":
            out_names.append(name)
            shape = tuple(alloc.tensor_shape)
            dtype = mybir.dt.np(alloc.dtype)
            out_avals.append(jax.core.ShapedArray(shape, dtype))
            zero_outs.append(np.zeros(shape, dtype))
    n_params = len(in_names)
    n_outs = len(out_avals)
    in_names_all = in_names + out_names
    if partition_name is not None:
        in_names_all = in_names_all + [partition_name]

    def _body(*args):
        operands = list(args)
        if partition_name is not None:
            operands.append(bass2jax.partition_id_tensor())
        outs = bass2jax._bass_exec_p.bind(
            *operands,
            out_avals=tuple(out_avals),
            in_names=tuple(in_names_all),
            out_names=tuple(out_names),
            lowering_input_output_aliases=(),
            sim_require_finite=True,
            sim_require_nnan=True,
            nc=nc,
        )
        return tuple(outs)

    devices = jax.devices()[:N_CORES]
    mesh = Mesh(np.asarray(devices), ("core",))
    in_specs = (PartitionSpec("core"),) * (n_params + n_outs)
    out_specs = (PartitionSpec("core"),) * len(out_names)
    sharded = jax.jit(
        shard_map(_body, mesh=mesh, in_specs=in_specs, out_specs=out_specs,
                  check_rep=False),
        keep_unused=True,
    )

    class Runner:
        pass

    r = Runner()
    r.nc = nc
    r.sharded = sharded
    r.in_names = in_names
    r.out_names = out_names
    r.zero_outs = zero_outs
    r.out_avals = out_avals
    return r


def _run_spmd(runner, in_maps):
    concat_in = [
        np.concatenate([np.asarray(in_maps[c][nm]) for c in range(N_CORES)],
                       axis=0)
        for nm in runner.in_names
    ]
    concat_zeros = [
        np.zeros((N_CORES * z.shape[0], *z.shape[1:]), z.dtype)
        for z in runner.zero_outs
    ]
    outs = runner.sharded(*concat_in, *concat_zeros)
    return [
        {nm: np.asarray(outs[i]).reshape(N_CORES,
                                         *runner.out_avals[i].shape)[c]
         for i, nm in enumerate(runner.out_names)}
        for c in range(N_CORES)
    ]


def _token_map():
    """TOK[c, p] = global token index held by core c at shard position p,
    under the per-512-group ReduceScatter layout."""
    TOK = np.empty((N_CORES, SSH), np.int64)
    for c in range(N_CORES):
        p = np.arange(SSH)
        g = p // 64
        TOK[c] = 512 * g + c * 64 + (p % 64)
    return TOK


_TOK = _token_map()


def _stage1_in_maps(inputs):
    x = np.ascontiguousarray(np.asarray(inputs["x"], np.float32)[0])
    g1 = np.asarray(inputs["ln1_g"], np.float32)
    b1v = np.asarray(inputs["ln1_b"], np.float32)
    g2 = np.asarray(inputs["ln2_g"], np.float32)
    b2v = np.asarray(inputs["ln2_b"], np.float32)
    Wq, bq = (np.asarray(inputs["Wq"], np.float32),
              np.asarray(inputs["bq"], np.float32))
    Wk, bk = (np.asarray(inputs["Wk"], np.float32),
              np.asarray(inputs["bk"], np.float32))
    Wv, bv = (np.asarray(inputs["Wv"], np.float32),
              np.asarray(inputs["bv"], np.float32))
    Wo, bo = (np.asarray(inputs["Wo"], np.float32),
              np.asarray(inputs["bo"], np.float32))
    Wr, br = (np.asarray(inputs["Wr"], np.float32),
              np.asarray(inputs["br"], np.float32))

    Wqf, bqf = g1[:, None] * Wq, bq + b1v @ Wq
    Wkf, bkf = g1[:, None] * Wk, bk + b1v @ Wk
    Wvf, bvf = g1[:, None] * Wv, bv + b1v @ Wv
    Wrf, brf = g2[:, None] * Wr, br + b2v @ Wr

    tri = np.triu(np.ones((P, P), np.float32))

    m = x.mean(axis=1)
    var = x.var(axis=1)
    std = np.sqrt(var + EPS)
    rstd = (1.0 / std).astype(np.float32)
    common = dict(
        xT=np.ascontiguousarray((x * rstd[:, None]).T.astype(np.float32)),
        srows_in=np.ascontiguousarray(
            np.stack([np.ones(S, np.float32), -m * rstd]).astype(np.float32)),
    )
    in_maps = []
    for c in range(N_CORES):
        cols = slice(c * HPC * HD, (c + 1) * HPC * HD)
        wqkv = np.concatenate([Wqf[:, cols], Wkf[:, cols], Wvf[:, cols]],
                              axis=1)
        bqkv = np.concatenate([bqf[cols], bkf[cols], bvf[cols]])
        wpack_r = np.concatenate(
            [np.eye(P, dtype=np.float32)] +
            [wqkv[d * P:(d + 1) * P, :] for d in range(ND)] +
            [Wo[cols, :], tri,
             np.full((P, 1), 1.0 / D, np.float32)], axis=1)
        corr = np.stack([bqkv, wqkv.sum(axis=0)])
        m = dict(common)
        m.update(
            wpack_r=np.ascontiguousarray(wpack_r.astype(np.float32)),
            corr=np.ascontiguousarray(corr.astype(np.float32)),
        )
        in_maps.append(m)
    return in_maps


def _pick_capacity(max_load):
    # device capacity is fixed at 512 (single PSUM column-split, minimal
    # instruction count); tokens beyond 512 per expert are computed on host.
    return 512


def _stage2_in_maps(h2, gates, inputs, C):
    g2 = np.asarray(inputs["ln2_g"], np.float32)
    b2v = np.asarray(inputs["ln2_b"], np.float32)
    e_w1 = np.asarray(inputs["e_w1"], np.float32)
    e_b1 = np.asarray(inputs["e_b1"], np.float32)
    e_w2 = np.asarray(inputs["e_w2"], np.float32)
    e_b2 = np.asarray(inputs["e_b2"], np.float32)

    in_maps, idxs = [], []
    for e in range(N_CORES):
        idx_full = np.nonzero(gates[:, e] > 0.0)[0]
        idxs.append(idx_full)
        idx = idx_full[:C]
        h2g = np.zeros((C, D), np.float32)
        h2g[:len(idx)] = h2[idx]
        gv = np.zeros((C,), np.float32)
        gv[:len(idx)] = gates[idx, e]
        w1f = (g2[:, None] * e_w1[e]).astype(np.float32)
        b1f = e_b1[e] + b2v @ e_w1[e]
        w1host = np.ascontiguousarray(
            (w1f * W1S).reshape(ND, P, NF, P).transpose(1, 2, 0, 3).reshape(
                P, NF * ND * P)).astype(NP_W1)
        w2host = np.ascontiguousarray(
            (e_w2[e] * W2S).reshape(NF, P, ND, P).transpose(1, 2, 0, 3)
            .reshape(P, ND * NF * P)).astype(NP_W2)
        h2gT_host = np.ascontiguousarray(
            h2g.T.reshape(ND, P, C).transpose(1, 0, 2).reshape(
                P, ND * C)).astype(NP_H2G)
        in_maps.append(dict(
            h2gT=h2gT_host,
            w1=w1host,
            b1=np.ascontiguousarray(b1f.reshape(NF, P).T.astype(np.float32)),
            w2=w2host,
            b2=np.ascontiguousarray(
                (e_b2[e] * W2S).reshape(ND, P).T.astype(np.float32)),
            gates=(gv / W2S)[None, :],
        ))
    return in_maps, idxs


def kernel(**inputs):
    r1 = _get_stage("s1")
    in_maps1 = _stage1_in_maps(inputs)
    res1 = _run_spmd(r1, in_maps1)

    # host post: residual add, LN2, fp32 router, top-2 softmax gates.
    # (These feed only the host-side expert gather, so they live here;
    # exact fp32 keeps the 2.3e-5 top-2 margins safe.)
    x = np.asarray(inputs["x"], np.float32)[0]
    bo = np.asarray(inputs["bo"], np.float32)
    order = _TOK.reshape(-1)
    a = np.empty((S, D), np.float32)
    a[order] = np.concatenate([res1[c]["rs_ext"] for c in range(N_CORES)])
    a += x + bo

    g2 = np.asarray(inputs["ln2_g"], np.float32)
    b2v = np.asarray(inputs["ln2_b"], np.float32)
    Wr = np.asarray(inputs["Wr"], np.float32)
    br = np.asarray(inputs["br"], np.float32)
    mu = a.mean(axis=1, keepdims=True)
    var = ((a - mu) ** 2).mean(axis=1, keepdims=True)
    h2 = ((a - mu) / np.sqrt(var + EPS)).astype(np.float32)
    logits = (h2 * g2 + b2v) @ Wr + br
    srt = np.sort(logits, axis=-1)
    kmask = logits >= srt[:, -2:-1]
    z = np.where(kmask, logits, -np.inf)
    z = z - z.max(axis=-1, keepdims=True)
    ez = np.exp(z)
    gates = (ez / ez.sum(axis=-1, keepdims=True)).astype(np.float32)

    loads = (gates > 0.0).sum(axis=0)
    C = _pick_capacity(loads.max())
    r2 = _get_stage("s2", C=C)
    in_maps2, idxs = _stage2_in_maps(h2, gates, inputs, C)
    res2 = _run_spmd(r2, in_maps2)

    out = a
    g2 = np.asarray(inputs["ln2_g"], np.float32)
    b2v = np.asarray(inputs["ln2_b"], np.float32)
    for e in range(N_CORES):
        idx = idxs[e]
        ndev = min(len(idx), C)
        out[idx[:ndev]] += np.asarray(res2[e]["outT"][:, :ndev],
                                      np.float32).T
        if len(idx) > ndev:
            # host tail: exact fp32 FFN for the few overflow tokens
            tidx = idx[ndev:]
            w1f = g2[:, None] * np.asarray(inputs["e_w1"], np.float32)[e]
            b1f = (np.asarray(inputs["e_b1"], np.float32)[e]
                   + b2v @ np.asarray(inputs["e_w1"], np.float32)[e])
            pre = h2[tidx] @ w1f + b1f
            try:
                from scipy.special import erf
            except ImportError:
                import math
                _erf = np.frompyfunc(math.erf, 1, 1)

                def erf(z):
                    return _erf(z).astype(np.float32)
            mid = pre * 0.5 * (1.0 + erf(pre / np.sqrt(2.0)))
            eo = (mid @ np.asarray(inputs["e_w2"], np.float32)[e]
                  + np.asarray(inputs["e_b2"], np.float32)[e])
            out[tidx] += eo * gates[tidx, e][:, None]
    return out.reshape(1, S, D).astype(np.float32)

